# revision 12
# baseline (speedup 1.0000x reference)
"""Trainium2 Bass kernel for DigitConvolutionalModel (dense_cnn).

Network: x[B,784] -> 3x3 valid conv (1 channel) -> flatten[676] -> 4-layer MLP
         (676->200 relu, 200->200 relu, 200->200 relu, 200->10).

Key algebraic fold: the conv is linear and feeds the first dense layer with
no nonlinearity in between (reshape only), so conv+W1 collapse into a single
effective weight W1p = C @ W1 of shape [784, 200], where C is the im2col
matrix of conv_w. The whole network becomes a 4-layer MLP:

    out = relu(relu(relu(x @ W1p + b1) @ W2 + b2) @ W3 + b3) @ W4 + b4

Sharding: pure data parallel over 8 cores (batch 65536 -> 8192/core).
On-device dataflow keeps activations feature-major ([features, batch]) so
every layer is a plain lhsT.T @ rhs matmul chain with no on-chip transposes.
The host pre-arranges each core's x shard as [NG*112, 7*512] so each batch
group of 512 samples loads with ONE dma_start whose per-partition runs are
fully contiguous (14 KB), and packs all weights+biases into one [128, WC]
blob loaded by one DMA. Outputs leave as [10, 8192] per core and the host
transposes them back.

Matmul dtype mode: default "fp16" (measured end-to-end relative error
~7e-4). fp16 streams the PE at the same 1 col/cycle as "f32r" but with
cheaper weight loads (2-byte LDWEIGHTS, partial FWL) and HALF the x HBM
traffic (12.8 MB/core/pass vs 25.7 — the per-NC HBM limit is ~358 GB/s,
so fp32 x alone had a ~72 us floor). x is cast to fp16 on the host
during packing; the weight blob stays fp32 and its matmul regions are
downcast once on device. A short PE pre-warm (dummy matmuls during the
initial weight DMA) releases the HAM clock throttle (1.2 -> 2.4 GHz)
before real work starts.

Modes: "fp16" (default), "bf16", "f32r" (original), "fp16d"/"bf16d"
(dual-group: weights loaded once per pair of groups, second group's
matmuls reuse the stationary via ldweights=False — measured neutral to
+9% depending on device power state, kept for experimentation).
"""

import numpy as np

import concourse.bacc as bacc
import concourse.mybir as mybir
import concourse.tile as tile
from concourse.bass_utils import run_bass_kernel_spmd

B = 65536
IMG = 28
KW = 3
CONV_OUT = (IMG - KW + 1) ** 2  # 676
HID = 200
OUT = 10
K1 = IMG * IMG  # 784

N_CORES = 8
BC = B // N_CORES  # 8192 rows per core
NB = 512  # batch columns per matmul group (one PSUM bank)
NG = BC // NB  # 16 groups
K1C = 112  # K1 split into 7 chunks of 112 (<=128)
NK1 = K1 // K1C  # 7
# 200 split for partition-dim chunks (both as matmul M and as next-layer K)
HCHUNKS = [(0, 128), (128, 72)]

F32 = mybir.dt.float32

# ---- weight-blob column layout (shared by host packer + device slicer) ----
W1_COL = 0                      # 7 chunks of [112, 200]
W2_COL = W1_COL + NK1 * HID     # 2 chunks of [128/72, 200]
W3_COL = W2_COL + 2 * HID
W4_COL = W3_COL + 2 * HID       # 2 chunks of [128/72, 10]
B_COL = W4_COL + 2 * OUT        # b1(2) b2(2) b3(2) cols, then b4
B4_COL = B_COL + 6
W4P_COL = B4_COL + 1            # 2 chunks of [128/72, 32]: W4 zero-padded to M=32
WC = W4P_COL + 2 * 32

_cache: dict = {}


def _build(mode: str, repeats: int = 1, xbufs: int = 3, hbufs: int = 2,
           obufs: int = 2, pack_l4: bool = False, hwloop: int | None = None):
    """Build + compile the per-core Bass program (same NEFF on all cores).

    repeats>1 re-runs the whole batch loop (same data) inside one NEFF —
    used only for benchmarking device time by slope.

    hwloop=N wraps the whole body in a For_i hardware loop with N trips —
    program size stays constant across N, so wall-clock slope over N
    isolates true device time (the axon tunnel hides anything smaller).

    pack_l4: run L4's M=10 matmuls in PE column group 3 (tile_position
    (0, 96)) concurrently with the last two L1m1 (M=72, col groups 0-2)
    matmuls — would hide L4's 2 matmul-times (~7 us/pass). DO NOT USE:
    this neuronxcc build rejects any matmul dst PSUM at non-zero base
    partition (ISA check 's3d3_mm_valid_dst_partition'), even 32-aligned
    full-strip slices. Kept for documentation.

    mode "fp16"/"bf16": x arrives 16-bit from the host (halves the
    HBM-bound x stream); the weight blob stays fp32 and its matmul
    regions are cast to 16-bit once on device (biases read fp32 from the
    blob as before).
    """
    is16 = mode in ("fp16", "bf16")
    if is16:
        DT = mybir.dt.float16 if mode == "fp16" else mybir.dt.bfloat16
        WDT = F32  # weight blob dtype (cast on device)
    else:
        DT = mybir.dt.float32r if mode == "f32r" else F32
        WDT = DT

    nc = bacc.Bacc("TRN2", target_bir_lowering=False, debug=False)

    xh = nc.dram_tensor("xh", [NG * K1C, NK1 * NB], DT, kind="ExternalInput")
    wb = nc.dram_tensor("wb", [128, WC], WDT, kind="ExternalInput")
    outT = nc.dram_tensor("outT", [OUT, BC], F32, kind="ExternalOutput")

    relu = mybir.ActivationFunctionType.Relu

    with tile.TileContext(nc) as tc:
        with (
            tc.tile_pool(name="wpool", bufs=1) as wpool,
            tc.tile_pool(name="xpool", bufs=xbufs) as xpool,
            tc.tile_pool(name="hpool", bufs=hbufs) as hpool,
            tc.tile_pool(name="opool", bufs=obufs) as opool,
            tc.tile_pool(name="psum", bufs=1, space="PSUM") as psum,
        ):
            # Weight load split across rings so it overlaps the first x-group
            # load: w1p columns (needed first) on the ACT HWDGE ring, the
            # rest (needed only from L2 on) via SWDGE. The SP HWDGE ring
            # stays a pure back-to-back stream of x-group loads.
            wt = wpool.tile([128, WC], WDT, tag="wt")
            nc.scalar.dma_start(out=wt[:, 0:W2_COL], in_=wb.ap()[:, 0:W2_COL])
            nc.gpsimd.dma_start(out=wt[:, W2_COL:WC], in_=wb.ap()[:, W2_COL:WC])
            if is16:
                # one-time downcast of the matmul weight regions
                wt16 = wpool.tile([128, B_COL], DT, tag="wt16")
                nc.scalar.copy(out=wt16[:, 0:W2_COL], in_=wt[:, 0:W2_COL])
                nc.scalar.copy(out=wt16[:, W2_COL:B_COL], in_=wt[:, W2_COL:B_COL])
                wmm = wt16
            else:
                wmm = wt

            def w1s(k, h0, hsz):  # lhsT [112, hsz] for L1 chunk k
                return wmm[0:K1C, W1_COL + k * HID + h0 : W1_COL + k * HID + h0 + hsz]

            def w23s(base, k, h0, hsz):  # lhsT [ksz, hsz] for L2/L3
                k0, ksz = HCHUNKS[k]
                c = base + k * HID + h0
                return wmm[0:ksz, c : c + hsz]

            def w4s(k):  # lhsT [ksz, 10]
                k0, ksz = HCHUNKS[k]
                c = W4_COL + k * OUT
                return wmm[0:ksz, c : c + OUT]

            def w4ps(k):  # lhsT [ksz, 32] (W4 zero-padded, for col-group 3)
                k0, ksz = HCHUNKS[k]
                c = W4P_COL + k * 32
                return wt[0:ksz, c : c + 32]

            def bs(idx, hsz):  # bias column [hsz, 1] as f32
                if is16:
                    return wt[0:hsz, B_COL + idx : B_COL + idx + 1]
                return wt[0:hsz, B_COL + idx : B_COL + idx + 1].bitcast(F32)

            # ---- emission helpers ----
            def l1_chunk(xg, i, l4_pair=None):
                h0, hsz = HCHUNKS[i]
                ps = psum.tile([hsz, NB], F32, tag=f"ps1_{i}")
                ps4 = None
                if l4_pair is not None:
                    hin3, g4 = l4_pair
                    ps4full = psum.tile([128, NB], F32, tag="ps4")
                    ps4 = ps4full[96:128, :]
                for k in range(NK1):
                    nc.tensor.matmul(
                        ps, w1s(k, h0, hsz), xg[:, k, :],
                        start=(k == 0), stop=(k == NK1 - 1),
                        skip_group_check=l4_pair is not None,
                    )
                    if l4_pair is not None and k >= NK1 - 2:
                        # L4 (M=10) in col group 3, concurrent with this
                        # M=72 matmul occupying col groups 0-2
                        kk = k - (NK1 - 2)
                        nc.tensor.matmul(
                            ps4, w4ps(kk), l4_pair[0][kk],
                            start=(kk == 0), stop=(kk == 1),
                            tile_position=(0, 96), skip_group_check=True,
                        )
                h = hpool.tile([hsz, NB], DT, tag=f"h1_{i}")
                nc.scalar.activation(h, ps, relu, bias=bs(i, hsz))
                if l4_pair is not None:
                    g4 = l4_pair[1]
                    o = opool.tile([128, NB], F32, tag="o")
                    nc.vector.tensor_scalar_add(
                        o[96 : 96 + OUT, :], ps4[0:OUT, :],
                        wt[96 : 96 + OUT, B4_COL : B4_COL + 1].bitcast(F32),
                    )
                    nc.scalar.dma_start(
                        out=outT.ap()[:, g4 * NB : (g4 + 1) * NB],
                        in_=o[96 : 96 + OUT, :],
                    )
                return h

            def dense(hin, base, li):  # L2 (li=2) / L3 (li=3) full layer
                hout = []
                for i, (h0, hsz) in enumerate(HCHUNKS):
                    ps = psum.tile([hsz, NB], F32, tag=f"ps{li}_{i}")
                    for k in range(len(HCHUNKS)):
                        nc.tensor.matmul(
                            ps, w23s(base, k, h0, hsz), hin[k],
                            start=(k == 0), stop=(k == len(HCHUNKS) - 1),
                        )
                    h = hpool.tile([hsz, NB], DT, tag=f"h{li}_{i}")
                    nc.scalar.activation(
                        h, ps, relu, bias=bs(2 * (li - 1) + i, hsz)
                    )
                    hout.append(h)
                return hout

            def l4_out(hin, g):
                ps = psum.tile([OUT, NB], F32, tag="ps4")
                for k in range(len(HCHUNKS)):
                    nc.tensor.matmul(
                        ps, w4s(k), hin[k],
                        start=(k == 0), stop=(k == len(HCHUNKS) - 1),
                    )
                o = opool.tile([OUT, NB], F32, tag="o")
                nc.vector.tensor_scalar_add(
                    o, ps, wt[0:OUT, B4_COL : B4_COL + 1].bitcast(F32)
                )
                # ACT HWDGE ring: keeps the SP ring a pure back-to-back
                # stream of x-group loads (no head-of-line blocking on the
                # late-produced outputs).
                nc.scalar.dma_start(out=outT.ap()[:, g * NB : (g + 1) * NB], in_=o)

            # ---- PE pre-warm: the HAM clock gate keeps the PE at 1.2 GHz
            # until it has been busy ~3.4 us. The PE would otherwise idle
            # during the initial weight DMA + cast and then pay the cold
            # window on real matmuls. ~9 dummy matmuls on a zeroed scratch
            # tile keep the PE busy from t~0 so the un-throttle fires
            # right as real work starts. Scratch PSUM uses the spare bank.
            if hwloop is None:
                warm = wpool.tile([128, NB], DT, tag="warm")
                nc.vector.memset(warm, 0.0)
                psw = psum.tile([128, NB], F32, tag="psw")
                for _ in range(9):
                    nc.tensor.matmul(psw, warm[:, 0:128], warm,
                                     start=True, stop=True,
                                     skip_group_check=True)

            # ---- main loop: groups software-pipelined with a 1-group skew.
            # PE stream per iteration: L1m0(t) | L3(t-1) | L1m1(t) | L4(t-1)
            # | L2(t) — the independent L1 matmuls hide the ACT latency of
            # the previous group's dependent L3/L4 chain.
            import contextlib

            loop_cm = (tc.For_i(0, hwloop, 1) if hwloop is not None
                       else contextlib.nullcontext())
            with loop_cm:
                h2_prev = None
                prev_g = None
                for t in range(NG * repeats):
                    g = t % NG
                    xg = xpool.tile([K1C, NK1, NB], DT, tag="xg")
                    src = xh.ap()[g * K1C : (g + 1) * K1C, :].rearrange(
                        "p (k b) -> p k b", k=NK1
                    )
                    if t == 0 and hwloop is None:
                        # per-k-chunk loads: the first matmul starts after
                        # one 224 KB chunk instead of the whole 1.6 MB group
                        for k in range(NK1):
                            nc.sync.dma_start(out=xg[:, k, :], in_=src[:, k, :])
                    else:
                        nc.sync.dma_start(out=xg, in_=src)
                    h1_0 = l1_chunk(xg, 0)
                    h3_prev = dense(h2_prev, W3_COL, 3) if h2_prev is not None else None
                    if pack_l4 and h3_prev is not None:
                        h1_1 = l1_chunk(xg, 1, l4_pair=(h3_prev, prev_g))
                    else:
                        h1_1 = l1_chunk(xg, 1)
                        if h3_prev is not None:
                            l4_out(h3_prev, prev_g)
                    h2_prev = dense([h1_0, h1_1], W2_COL, 2)
                    prev_g = g
                # epilogue: finish the last group
                l4_out(dense(h2_prev, W3_COL, 3), prev_g)

    nc.compile()
    return nc


def _build_dual(mode: str, hwloop: int | None = None, xbufs: int = 3,
                hbufs: int = 2, obufs: int = 2):
    """Dual-group variant: each weight chunk is loaded into the PE once
    per PAIR of batch groups; the second group's matmul reuses the
    loaded stationary (ldweights=False), halving weight-load overhead.
    16-bit only (f32r matmuls cannot skip their weight reload).

    PSUM tags (8 banks exactly): ps1A/B (L1, reused m0->m1), ps2A/B,
    ps3A/B (reused across chunks), ps4A/B.
    """
    assert mode in ("fp16", "bf16")
    DT = mybir.dt.float16 if mode == "fp16" else mybir.dt.bfloat16
    NP2 = NG // 2

    nc = bacc.Bacc("TRN2", target_bir_lowering=False, debug=False)
    xh = nc.dram_tensor("xh", [NG * K1C, NK1 * NB], DT, kind="ExternalInput")
    wb = nc.dram_tensor("wb", [128, WC], F32, kind="ExternalInput")
    outT = nc.dram_tensor("outT", [OUT, BC], F32, kind="ExternalOutput")
    relu = mybir.ActivationFunctionType.Relu

    with tile.TileContext(nc) as tc:
        with (
            tc.tile_pool(name="wpool", bufs=1) as wpool,
            tc.tile_pool(name="xpool", bufs=xbufs) as xpool,
            tc.tile_pool(name="hpool", bufs=hbufs) as hpool,
            tc.tile_pool(name="opool", bufs=obufs) as opool,
            tc.tile_pool(name="psum", bufs=1, space="PSUM") as psum,
        ):
            wt = wpool.tile([128, WC], F32, tag="wt")
            nc.scalar.dma_start(out=wt[:, 0:W2_COL], in_=wb.ap()[:, 0:W2_COL])
            nc.gpsimd.dma_start(out=wt[:, W2_COL:WC], in_=wb.ap()[:, W2_COL:WC])
            wt16 = wpool.tile([128, B_COL], DT, tag="wt16")
            nc.scalar.copy(out=wt16[:, 0:W2_COL], in_=wt[:, 0:W2_COL])
            nc.scalar.copy(out=wt16[:, W2_COL:B_COL], in_=wt[:, W2_COL:B_COL])

            def w1s(k, h0, hsz):
                return wt16[0:K1C,
                            W1_COL + k * HID + h0 : W1_COL + k * HID + h0 + hsz]

            def w23s(base, k, h0, hsz):
                _, ksz = HCHUNKS[k]
                c = base + k * HID + h0
                return wt16[0:ksz, c : c + hsz]

            def w4s(k):
                _, ksz = HCHUNKS[k]
                c = W4_COL + k * OUT
                return wt16[0:ksz, c : c + OUT]

            def bs(idx, hsz):
                return wt[0:hsz, B_COL + idx : B_COL + idx + 1]

            def mm_pair(psA, psB, w, mvA, mvB, start, stop):
                nc.tensor.matmul(psA, w, mvA, start=start, stop=stop,
                                 skip_group_check=True)
                i2 = nc.tensor.matmul(psB, w, mvB, start=start, stop=stop,
                                      skip_group_check=True)
                i2.ldweights = False

            def l1_pair(xg, i):
                h0, hsz = HCHUNKS[i]
                psA = psum.tile([hsz, NB], F32, tag="ps1A", name=f"ps1A_{i}")
                psB = psum.tile([hsz, NB], F32, tag="ps1B", name=f"ps1B_{i}")
                for k in range(NK1):
                    mm_pair(psA, psB, w1s(k, h0, hsz),
                            xg[:, 0, k, :], xg[:, 1, k, :],
                            k == 0, k == NK1 - 1)
                hA = hpool.tile([hsz, NB], DT, tag=f"h1_{i}A", name=f"h1_{i}A")
                hB = hpool.tile([hsz, NB], DT, tag=f"h1_{i}B", name=f"h1_{i}B")
                nc.scalar.activation(hA, psA, relu, bias=bs(i, hsz))
                nc.scalar.activation(hB, psB, relu, bias=bs(i, hsz))
                return hA, hB

            def dense_pair(hin, base, li):
                out = []
                for i, (h0, hsz) in enumerate(HCHUNKS):
                    psA = psum.tile([hsz, NB], F32, tag=f"ps{li}A",
                                    name=f"ps{li}A_{i}")
                    psB = psum.tile([hsz, NB], F32, tag=f"ps{li}B",
                                    name=f"ps{li}B_{i}")
                    for k in range(len(HCHUNKS)):
                        mm_pair(psA, psB, w23s(base, k, h0, hsz),
                                hin[k][0], hin[k][1],
                                k == 0, k == len(HCHUNKS) - 1)
                    hA = hpool.tile([hsz, NB], DT, tag=f"h{li}_{i}A",
                                    name=f"h{li}_{i}A")
                    hB = hpool.tile([hsz, NB], DT, tag=f"h{li}_{i}B",
                                    name=f"h{li}_{i}B")
                    nc.scalar.activation(hA, psA, relu,
                                         bias=bs(2 * (li - 1) + i, hsz))
                    nc.scalar.activation(hB, psB, relu,
                                         bias=bs(2 * (li - 1) + i, hsz))
                    out.append((hA, hB))
                return out

            def l4_pair(hin, gA):
                psA = psum.tile([OUT, NB], F32, tag="ps4A", name="ps4A")
                psB = psum.tile([OUT, NB], F32, tag="ps4B", name="ps4B")
                for k in range(len(HCHUNKS)):
                    mm_pair(psA, psB, w4s(k), hin[k][0], hin[k][1],
                            k == 0, k == len(HCHUNKS) - 1)
                for s, ps in ((0, psA), (1, psB)):
                    o = opool.tile([OUT, NB], F32, tag=f"o{s}", name=f"o{s}")
                    nc.vector.tensor_scalar_add(
                        o, ps, wt[0:OUT, B4_COL : B4_COL + 1])
                    g = gA + s
                    nc.scalar.dma_start(out=outT.ap()[:, g * NB : (g + 1) * NB],
                                        in_=o)

            import contextlib

            loop_cm = (tc.For_i(0, hwloop, 1) if hwloop is not None
                       else contextlib.nullcontext())
            with loop_cm:
                h2_prev = None
                prev_gA = None
                for j in range(NP2):
                    gA = 2 * j
                    xg = xpool.tile([K1C, 2, NK1, NB], DT, tag="xg",
                                    name=f"xg{j}")
                    for s in range(2):
                        src = xh.ap()[(gA + s) * K1C : (gA + s + 1) * K1C, :]
                        nc.sync.dma_start(
                            out=xg[:, s],
                            in_=src.rearrange("p (k b) -> p k b", k=NK1))
                    h1_0 = l1_pair(xg, 0)
                    h3_prev = (dense_pair(h2_prev, W3_COL, 3)
                               if h2_prev is not None else None)
                    h1_1 = l1_pair(xg, 1)
                    if h3_prev is not None:
                        l4_pair(h3_prev, prev_gA)
                    h2_prev = dense_pair([h1_0, h1_1], W2_COL, 2)
                    prev_gA = gA
                l4_pair(dense_pair(h2_prev, W3_COL, 3), prev_gA)

    nc.compile()
    return nc


def _im2col(conv_w: np.ndarray) -> np.ndarray:
    """C[784, 676] with h_conv = x @ C (cross-correlation, valid)."""
    co = IMG - KW + 1
    C = np.zeros((IMG * IMG, co * co), dtype=np.float64)
    ii, jj = np.meshgrid(np.arange(co), np.arange(co), indexing="ij")
    q = (ii * co + jj).ravel()
    for di in range(KW):
        for dj in range(KW):
            p = ((ii + di) * IMG + (jj + dj)).ravel()
            C[p, q] += conv_w[di, dj]
    return C


def _pack_weights(W1p, b1, W2, b2, W3, b3, W4, b4) -> np.ndarray:
    wb = np.zeros((128, WC), dtype=np.float32)
    for k in range(NK1):
        wb[0:K1C, W1_COL + k * HID : W1_COL + (k + 1) * HID] = W1p[
            k * K1C : (k + 1) * K1C
        ]
    for i, (h0, hsz) in enumerate(HCHUNKS):
        wb[0:hsz, W2_COL + i * HID : W2_COL + (i + 1) * HID] = W2[h0 : h0 + hsz]
        wb[0:hsz, W3_COL + i * HID : W3_COL + (i + 1) * HID] = W3[h0 : h0 + hsz]
        wb[0:hsz, W4_COL + i * OUT : W4_COL + (i + 1) * OUT] = W4[h0 : h0 + hsz]
        wb[0:hsz, W4P_COL + i * 32 : W4P_COL + i * 32 + OUT] = W4[h0 : h0 + hsz]
        wb[0:hsz, B_COL + i] = b1[h0 : h0 + hsz]
        wb[0:hsz, B_COL + 2 + i] = b2[h0 : h0 + hsz]
        wb[0:hsz, B_COL + 4 + i] = b3[h0 : h0 + hsz]
    wb[0:OUT, B4_COL] = b4
    wb[96 : 96 + OUT, B4_COL] = b4  # copy at partition 96 for the packed-L4 path
    return wb


def _pack_x(x_shard: np.ndarray, mode: str = "f32r") -> np.ndarray:
    """[8192, 784] -> [NG*112, 7*512]: row g*112+p holds, for each k-chunk,
    the 512 batch values of pixel k*112+p in group g (contiguous per row)."""
    # xT[k*112+p, g*512+b] -> xh[g, p, k, b]
    xt = x_shard.T.reshape(NK1, K1C, NG, NB)  # [k, p, g, b]
    xh = np.ascontiguousarray(xt.transpose(2, 1, 0, 3))  # [g, p, k, b]
    xh = xh.reshape(NG * K1C, NK1 * NB)
    if mode in ("fp16", "fp16d"):
        return xh.astype(np.float16)
    if mode in ("bf16", "bf16d"):
        import ml_dtypes

        return xh.astype(ml_dtypes.bfloat16)
    return xh


def kernel(x, conv_w, W1, b1, W2, b2, W3, b3, W4, b4, _mode="fp16"):
    x = np.asarray(x, dtype=np.float32)
    C = _im2col(np.asarray(conv_w, dtype=np.float64))
    W1p = (C @ np.asarray(W1, dtype=np.float64)).astype(np.float32)

    if _mode not in _cache:
        if _mode == "f32rp":
            _cache[_mode] = _build("f32r", pack_l4=True)
        elif _mode in ("fp16d", "bf16d"):
            _cache[_mode] = _build_dual(_mode[:-1])
        else:
            _cache[_mode] = _build(_mode)
    nc = _cache[_mode]

    wb = _pack_weights(
        W1p,
        np.asarray(b1, np.float32), np.asarray(W2, np.float32),
        np.asarray(b2, np.float32), np.asarray(W3, np.float32),
        np.asarray(b3, np.float32), np.asarray(W4, np.float32),
        np.asarray(b4, np.float32),
    )
    in_maps = []
    for c in range(N_CORES):
        in_maps.append({"xh": _pack_x(x[c * BC : (c + 1) * BC], _mode), "wb": wb})

    res = run_bass_kernel_spmd(nc, in_maps, core_ids=list(range(N_CORES)))

    out = np.empty((B, OUT), dtype=np.float32)
    for c in range(N_CORES):
        out[c * BC : (c + 1) * BC] = res.results[c]["outT"].T
    return out

